# revision 43
# baseline (speedup 1.0000x reference)
"""Causal multi-head attention block on 8 Trainium2 NeuronCores.

Problem: x[4,2048,1024] -> QKV proj (16 heads, dh=64) -> causal softmax
attention -> out proj. Sharding: core = (batch, head-half): each core
computes QKV for 8 heads of one batch, flash-style attention for those
heads, and a partial O-projection over its 512 W_o input columns; the
host sums the two partials per batch (tensor-parallel unshard).

Device kernel (identical SPMD program, per-core data). Structure:
  - Q/K projection runs as fp8(e4m3) DoubleRow matmuls: weights are
    host-prescaled by 64 (to clear the e4m3 denormal range) and the
    1/64^2 is folded into the softmax exp scale. Contraction 1024 is
    4 DR matmuls of virtual-K 256 ([128 parts, 2 kd-chunks, .] APs on
    the natural [p, kd, t] tile layout -- no data shuffles). The fp8
    x.T copy is converted on-device by the (early-idle) vector engine.
    V and the O-projection stay bf16 (their error lands on the output).
  - x.T ships as four token-chunk tensors so chunk 0 (all of what the
    first q-chunk needs) streams first; HBM-in is the startup critical
    path (~358 GB/s shared across queues), so the first-needed 3MB
    (W_qk8 | x.T chunk 0 | W_v) leads on three separate queues.
  - scores are computed transposed, S.T[k_tile, q_span] = K.T_blk^T@Q.T.
    Each score call covers ONE k-tile for BOTH heads of a pair (even
    parity head on partitions 0:64, odd on 64:128 -> the two matmuls
    run concurrently as row-tiled PE ops) packed at column 0/512 of one
    2-bank PSUM tile; ScalarE exps both with a single ACTIVATE
    (contiguous for full tiles, 2-segment strided for diagonal ones).
    Diagonal blocks are masked after exp with a 0/1 triangle multiply.
  - O.T[c, q] accumulates with V' stationary: V' has 64 V columns and
    64 ones-columns (parity-dependent order) so the matmul broadcasts
    the softmax denominator for free; normalization is one
    reciprocal_approx_fast + one cross-partition-base multiply.
  - Loop order is J-outer (q-chunk), head-pair-inner. QKV / O-proj
    units pace into the attention phase as PE filler, with the
    deferrable O-projection pushed into the late, ScalarE-bound chunks
    (J=3 attends 4x the keys of J=0). Fill units draw from their own
    2-buf PSUM pool so they never steal a score-pipeline buffer. The
    post-loop O-proj tail runs at 256-column granularity across both
    free PSUM pools with stores spread over two DMA queues.
"""

import numpy as np
import ml_dtypes

BF16 = ml_dtypes.bfloat16
F8E4 = ml_dtypes.float8_e4m3

B, T, D = 4, 2048, 1024
NH, DH = 16, 64
HPC = 8            # heads per core
OC = HPC * DH      # 512: per-core head columns
NT = T // 128      # 16 q/k tiles of 128
ND = D // 128      # 8 d-tiles
N_CORES = 8
WSCALE = 64.0      # host pre-scale on W_qk/b_qk (e4m3 denormal dodge)

_cache = {}


def _build():
    import concourse.mybir as mybir
    import concourse.tile as tile
    from concourse import bacc

    f32 = mybir.dt.float32
    bf16 = mybir.dt.bfloat16
    fp8 = mybir.dt.float8e4
    Exp = mybir.ActivationFunctionType.Exp
    DR = mybir.MatmulPerfMode.DoubleRow

    nc = bacc.Bacc("TRN2", target_bir_lowering=False, debug=False,
                   num_devices=N_CORES)

    # xt halves: 1024-token chunks keep 2KB/partition DMA segments;
    # w8/wv ship (p n)-packed so each partition reads one contiguous run
    xt = [nc.declare_dram_parameter(f"xt{c}", [D, 1024], bf16, isOutput=False)
          for c in range(2)]
    w8 = nc.declare_dram_parameter("w8", [D, 2 * OC], fp8, isOutput=False)
    wv = nc.declare_dram_parameter("wvT", [D, OC], bf16, isOutput=False)
    wo = nc.declare_dram_parameter("woT", [OC, D], bf16, isOutput=False)
    bqk = nc.declare_dram_parameter("bqk", [128, 2 * OC // 128], f32, isOutput=False)
    bv = nc.declare_dram_parameter("bv", [128, OC], f32, isOutput=False)
    bo = nc.declare_dram_parameter("bo", [128, D], f32, isOutput=False)
    tri = nc.declare_dram_parameter("tri", [128, 128], bf16, isOutput=False)
    out = nc.declare_dram_parameter("out", [T, D], bf16, isOutput=True)

    with tile.TileContext(nc) as tc:
        with (
            tc.tile_pool(name="persist", bufs=1) as persist,
            tc.tile_pool(name="pt", bufs=8) as ptp,
            tc.tile_pool(name="dn", bufs=6) as dnp,
            tc.tile_pool(name="ostage", bufs=4) as ostage,
            tc.tile_pool(name="psS", bufs=2, space="PSUM") as psS,
            tc.tile_pool(name="psF", bufs=2, space="PSUM") as psF,
            tc.tile_pool(name="psO", bufs=2, space="PSUM") as psO,
        ):
            # ---- persistent SBUF tensors ----
            XT = persist.tile([128, ND, T], bf16)          # x.T d-tiles (V path)
            X8 = persist.tile([128, ND, T], fp8)           # x.T e4m3 (QK path)
            W8 = persist.tile([128, ND, 2 * OC], fp8)      # 64*W_qk.T e4m3
            WV = persist.tile([128, ND, OC], bf16)
            WO = persist.tile([128, OC // 128, D], bf16)
            BQK = persist.tile([128, 2 * OC // 128], f32)
            BV = persist.tile([128, OC], f32)
            BO = persist.tile([128, D], f32)
            TRI = persist.tile([128, 128], bf16)
            QKT = persist.tile([128, ND, T], bf16)         # [o, t] 64*(Q.T|K.T)
            # V' per head, 128 cols: even h: [V(64) | 1*64]; odd h:
            # [1*64 | V(64)]. O.T rows land on partitions (h%2)*64..+64 and
            # the other 64 rows all become the softmax denominator.
            VP = persist.tile([128, NT, HPC, 128], bf16)
            # attn out.T [c, q] -- one tile per q-chunk J so the deferred
            # O-projection of chunk J never waits on later chunks' writes
            OTJ = [persist.tile([128, OC // 128, 512], bf16, name=f"OTJ{j}")
                   for j in range(4)]

            # warm-up junk matmuls: keep the PE HAM clock-gate warm while
            # the first input DMA chunks stream in; results never read.
            JNK = persist.tile([128, 512], bf16)
            nc.vector.memset(JNK[:], 0.5)
            for g in range(5):
                jps = psF.tile([128, 512], f32, tag="f", name=f"jnk{g}")
                for m in range(8):
                    nc.tensor.matmul(
                        jps[:], lhsT=JNK[:, 0:128], rhs=JNK[:],
                        start=(m == 0), stop=(m == 7),
                    )

            # ---- input DMA: first-needed first, balanced queues ----
            # J=0's critical 4MB (x.T lo, W8, W_v) splits evenly over the
            # sync/gpsimd queues by kd- or partition-halves (both keep the
            # per-partition segments contiguous), so it all lands together
            # ~8us after the DMA queues open; x.T hi and W_o follow.
            # two-queue cascade ordered strictly by first use; the scalar
            # queue carries no input so the exp stream never competes.
            # bv/bo ship pre-broadcast -- a [1,.]->[128,.] broadcast DMA
            # shreds into 128 tiny packets and stalls the shared engines.
            # three queues (per-queue bandwidth caps ~140GB/s; the gpsimd
            # queue consistently opens ~3.5us late, so the J=0-critical
            # x.T-lo halves ride sync+scalar)
            xtr = [t.rearrange("(n p) t -> p n t", p=128) for t in xt]
            w8r = w8.rearrange("(p n) o -> p n o", p=128)
            wvr = wv.rearrange("(p n) o -> p n o", p=128)
            wor = wo.rearrange("(n p) o -> p n o", p=128)
            nc.sync.dma_start(out=XT[:, 0:4, 0:1024], in_=xtr[0][:, 0:4, :])
            nc.sync.dma_start(out=W8[0:64], in_=w8r[0:64])
            nc.sync.dma_start(out=WV[0:64], in_=wvr[0:64])
            nc.sync.dma_start(out=XT[:, :, 1024:2048], in_=xtr[1])
            # the gpsimd-issued queue consistently opens ~3.5us late, so
            # the second J=0-critical x.T half leads the scalar queue
            nc.scalar.dma_start(out=XT[:, 4:8, 0:1024], in_=xtr[0][:, 4:8, :])
            nc.gpsimd.dma_start(out=W8[64:128], in_=w8r[64:128])
            nc.gpsimd.dma_start(out=WV[64:128], in_=wvr[64:128])
            nc.gpsimd.dma_start(out=WO[:], in_=wor)
            nc.scalar.dma_start(out=BQK[:], in_=bqk[:, :])
            nc.scalar.dma_start(out=TRI[:], in_=tri[:, :])
            nc.scalar.dma_start(out=BV[:], in_=bv[:, :])
            nc.scalar.dma_start(out=BO[:], in_=bo[:, :])
            # ones-columns of V' via the (otherwise idle) gpsimd engine --
            # these strided memsets cost 3.5us each and would delay the
            # fp8 cast on the vector queue
            nc.gpsimd.memset(VP[:, :, 0:HPC:2, DH:128], 1.0)
            nc.gpsimd.memset(VP[:, :, 1:HPC:2, 0:DH], 1.0)

            def emit_cv(tch):
                # vector-engine bf16 -> e4m3 convert of one x.T t-chunk
                nc.vector.tensor_copy(
                    X8[:, :, tch * 512:(tch + 1) * 512],
                    XT[:, :, tch * 512:(tch + 1) * 512],
                )

            emit_cv(0)

            # ---- QKV / O-proj units (PE filler) ----
            def emit_qk(ot, tch):
                # one [o, t] chunk: [128 o, 512 t] = 64*W_qk @ x.T + 64*b,
                # fp8 DoubleRow: 4 matmuls of virtual-K 256
                ps = psF.tile([128, 512], f32, tag="f",
                              name=f"qk{ot}_{tch}")
                for k in range(4):
                    nc.tensor.matmul(
                        ps[:],
                        lhsT=W8[:, 2 * k:2 * k + 2, ot * 128:(ot + 1) * 128],
                        rhs=X8[:, 2 * k:2 * k + 2, tch * 512:(tch + 1) * 512],
                        start=(k == 0), stop=(k == 3),
                        perf_mode=DR,
                    )
                nc.vector.tensor_scalar_add(
                    QKT[:, ot, tch * 512:(tch + 1) * 512], ps[:],
                    BQK[:, ot:ot + 1],
                )

            def emit_v(tt):
                # one [t, o] tile of V = x @ W_v.T + b, into parity layout
                ps = psF.tile([128, 512], f32, tag="f", name=f"v{tt}")
                for kd in range(ND):
                    nc.tensor.matmul(
                        ps[:],
                        lhsT=XT[:, kd, tt * 128:(tt + 1) * 128],
                        rhs=WV[:, kd, :],
                        start=(kd == 0), stop=(kd == ND - 1),
                    )
                nc.vector.tensor_tensor(
                    out=VP[:, tt, 0:HPC:2, 0:DH],
                    in0=ps[:].rearrange("p (a b) -> p a b", b=DH)[:, 0:HPC:2, :],
                    in1=BV[:].rearrange("p (a b) -> p a b", b=DH)[:, 0:HPC:2, :],
                    op=mybir.AluOpType.add,
                )
                nc.vector.tensor_tensor(
                    out=VP[:, tt, 1:HPC:2, DH:2 * DH],
                    in0=ps[:].rearrange("p (a b) -> p a b", b=DH)[:, 1:HPC:2, :],
                    in1=BV[:].rearrange("p (a b) -> p a b", b=DH)[:, 1:HPC:2, :],
                    op=mybir.AluOpType.add,
                )

            odma = [0]

            def emit_oproj(tq, oc2, pool=None, width=512, beng=None, dq=None):
                # out[tq, oc2-chunk] = O @ WoT + 0.5 b_o (partial over this
                # core's 512 W_o input columns)
                pool = pool or psF
                ps = pool.tile([128, width], f32,
                               tag="s" if pool is psS else "f",
                               name=f"op{tq}_{oc2}_{width}")
                lo = oc2 * width
                OT = OTJ[tq // 4]
                tql = tq % 4
                for ct in range(OC // 128):
                    nc.tensor.matmul(
                        ps[:],
                        lhsT=OT[:, ct, tql * 128:(tql + 1) * 128],
                        rhs=WO[:, ct, lo:lo + width],
                        start=(ct == 0), stop=(ct == OC // 128 - 1),
                    )
                ob = ostage.tile([128, width], bf16, tag="ob")
                (beng or nc.vector).tensor_tensor(
                    out=ob[:], in0=ps[:], in1=BO[:, lo:lo + width],
                    op=mybir.AluOpType.add,
                )
                q = dq or (nc.sync if odma[0] % 2 == 0 else nc.gpsimd)
                odma[0] += 1
                q.dma_start(
                    out=out[tq * 128:(tq + 1) * 128, lo:lo + width],
                    in_=ob[:],
                )

            # prologue (overlaps the input DMA): everything chunk J=0 needs
            for u in [("qk", 0, 0), ("qk", 4, 0), ("v", 0), ("qk", 1, 0),
                      ("qk", 5, 0), ("v", 1), ("qk", 2, 0), ("qk", 6, 0),
                      ("v", 2), ("qk", 3, 0), ("qk", 7, 0), ("v", 3)]:
                if u[0] == "v":
                    emit_v(u[1])
                else:
                    emit_qk(u[1], u[2])

            # fill schedule keyed by global pop index (one pop per score
            # call == one k-tile). (J,hp) starts at pop 16*J*(J+1)/2...
            # J bases 0/16/48/96; (J,hp) at base + hp*(J+1)*4. Deadlines:
            # qk(.,tc) before (tc,hp) scores; v(4J..4J+3) before (J,hp0)
            # PV; oproj(tq) after chunk tq//4 completes -- deferred into
            # the ScalarE-bound J=2/J=3 stretches.
            sched = {
                # J=0 (pops 0..15)
                2: [("cv", 1)],
                4: [("qk", 0, 1)], 6: [("qk", 4, 1)],
                8: [("v", 4)], 10: [("v", 5)], 12: [("v", 6)], 14: [("v", 7)],
                # J=1 (pops 16..47)
                16: [("cv", 2)],
                17: [("qk", 1, 1)], 19: [("qk", 5, 1)],
                20: [("cv", 3)],
                22: [("qk", 2, 1)], 24: [("qk", 6, 1)],
                27: [("qk", 3, 1)], 29: [("qk", 7, 1)],
                32: [("qk", 0, 2)], 34: [("qk", 4, 2)],
                36: [("v", 8)], 38: [("v", 9)], 40: [("v", 10)], 42: [("v", 11)],
                44: [("op", 0, 0)], 46: [("op", 0, 1)],
                # J=2 (pops 48..95): QK/V prefetch only (oproj deferred)
                50: [("qk", 1, 2)], 53: [("qk", 5, 2)],
                62: [("qk", 2, 2)], 65: [("qk", 6, 2)],
                74: [("qk", 3, 2)], 77: [("qk", 7, 2)],
                80: [("v", 12)], 83: [("v", 13)],
                85: [("qk", 0, 3)], 88: [("qk", 4, 3)],
                90: [("v", 14)], 93: [("v", 15)],
                # J=3 (pops 96..159): remaining qk + all deferred oproj
                96: [("op", 1, 0)], 98: [("qk", 1, 3)],
                100: [("op", 1, 1)], 102: [("qk", 5, 3)],
                104: [("op", 2, 0)], 106: [("op", 2, 1)],
                108: [("op", 3, 0)], 110: [("qk", 2, 3)],
                112: [("op", 3, 1)], 114: [("qk", 6, 3)],
                116: [("op", 4, 0)], 118: [("op", 4, 1)],
                120: [("op", 5, 0)], 122: [("qk", 3, 3)],
                124: [("op", 5, 1)], 126: [("qk", 7, 3)],
                128: [("op", 6, 0)], 130: [("op", 6, 1)],
                132: [("op", 7, 0)], 134: [("op", 7, 1)],
                138: [("op", 8, 0)], 141: [("op", 8, 1)],
                144: [("op", 9, 0)], 147: [("op", 9, 1)],
                150: [("op", 10, 0)], 153: [("op", 10, 1)],
                156: [("op", 11, 0)], 158: [("op", 11, 1)],
            }
            giter = [0]

            def pop_fill():
                g = giter[0]
                giter[0] += 1
                for u in sched.get(g, []):
                    if u[0] == "v":
                        emit_v(u[1])
                    elif u[0] == "qk":
                        emit_qk(u[1], u[2])
                    elif u[0] == "cv":
                        emit_cv(u[1])
                    else:
                        emit_oproj(u[1], u[2])

            # ---- attention; O.T accumulated with V' stationary ----
            # per call: one k-tile, both heads (h0 on partitions 0:64, h1
            # on 64:128 -> concurrent row-tiled matmuls), exp'd in one
            # ACTIVATE. av() lags one call behind the exp.
            def st_exp(h0, h1, J, i):
                ps = psS.tile([128, 1024], f32, tag="s",
                              name=f"ps{h0}_{J}_{i}")
                pt = ptp.tile([128, 1024], bf16, tag="p",
                              name=f"pt{h0}_{J}_{i}")
                QT0 = QKT[0:64, h0 // 2, :]
                KT0 = QKT[0:64, 4 + h0 // 2, :]
                QT1 = QKT[64:128, h1 // 2, :]
                KT1 = QKT[64:128, 4 + h1 // 2, :]
                qlo = max(J * 512, i * 128)
                span = (J + 1) * 512 - qlo
                nc.tensor.matmul(
                    ps[:, 0:span],
                    lhsT=KT0[:, i * 128:(i + 1) * 128],
                    rhs=QT0[:, qlo:qlo + span],
                    start=True, stop=True,
                )
                nc.tensor.matmul(
                    ps[:, 512:512 + span],
                    lhsT=KT1[:, i * 128:(i + 1) * 128],
                    rhs=QT1[:, qlo:qlo + span],
                    start=True, stop=True,
                )
                # QKT holds 64*(Q|K): scores are 4096x -> fold into scale
                sc = 0.125 / (WSCALE * WSCALE)
                if span == 512:
                    nc.scalar.activation(
                        out=pt[:], in_=ps[:], func=Exp, scale=sc)
                else:
                    psv = ps[:].rearrange("p (a b) -> p a b", b=512)[:, :, 0:span]
                    ptv = pt[:].rearrange("p (a b) -> p a b", b=512)[:, :, 0:span]
                    nc.scalar.activation(
                        out=ptv, in_=psv, func=Exp, scale=sc)
                return (pt, i, qlo, span)

            def av2(h0, h1, J, unit, otr0, otr1):
                pt, i, qlo, span = unit
                if i >= 4 * J:  # diagonal: zero upper triangle
                    nc.vector.tensor_tensor(
                        out=pt[:, 0:128], in0=pt[:, 0:128], in1=TRI[:],
                        op=mybir.AluOpType.mult,
                    )
                    nc.vector.tensor_tensor(
                        out=pt[:, 512:640], in0=pt[:, 512:640], in1=TRI[:],
                        op=mybir.AluOpType.mult,
                    )
                qloc = qlo - J * 512
                nc.tensor.matmul(
                    otr0[:, qloc:512],
                    lhsT=VP[:, i, h0, :],
                    rhs=pt[:, 0:span],
                    start=(i == 0), stop=(i == 4 * J + 3),
                )
                nc.tensor.matmul(
                    otr1[:, qloc:512],
                    lhsT=VP[:, i, h1, :],
                    rhs=pt[:, 512:512 + span],
                    start=(i == 0), stop=(i == 4 * J + 3),
                )

            def normalize(h, J, otr, last=False):
                # O.T rows (base prow) times 1/den rows (base drow; all 64
                # denominator rows are identical by construction)
                prow = (h % 2) * 64
                drow = 64 - prow
                rd = dnp.tile([128, 512], f32, tag="d", name=f"rd{h}_{J}")
                if drow == 0:
                    nc.vector.reciprocal_approx_fast(
                        rd[0:64, :], otr[0:64, :])
                else:
                    # reciprocal_approx_fast needs its source at base 0;
                    # the final chunk's copy runs on the (then-idle)
                    # scalar engine so the tail isn't DVE-serialized
                    rdc = dnp.tile([128, 512], f32, tag="dc",
                                   name=f"rdc{h}_{J}")
                    if last:
                        nc.scalar.copy(rdc[0:64, :], otr[drow:drow + 64, :])
                    else:
                        nc.vector.tensor_copy(
                            rdc[0:64, :], otr[drow:drow + 64, :])
                    nc.vector.reciprocal_approx_fast(
                        rd[0:64, :], rdc[0:64, :])
                nc.vector.tensor_tensor(
                    out=OTJ[J][prow:prow + 64, h // 2, :],
                    in0=otr[prow:prow + 64, :],
                    in1=rd[0:64, :],
                    op=mybir.AluOpType.mult,
                )

            for J in range(T // 512):
                for hp in range(HPC // 2):
                    h0, h1 = 2 * hp, 2 * hp + 1
                    otr0 = psO.tile([128, 512], f32, tag="o",
                                    name=f"otr{h0}_{J}")
                    otr1 = psO.tile([128, 512], f32, tag="o",
                                    name=f"otr{h1}_{J}")
                    prev = None
                    for i in range(4 * J + 4):
                        unit = st_exp(h0, h1, J, i)
                        pop_fill()
                        if prev is not None:
                            av2(h0, h1, J, prev, otr0, otr1)
                        prev = unit
                    av2(h0, h1, J, prev, otr0, otr1)
                    # h0's normalize (DVE chain) overlaps h1's PV matmuls
                    normalize(h0, J, otr0)
                    normalize(h1, J, otr1, last=(J == 3 and hp == 3))

            # tail: remaining O-projection at 256-col granularity, psum
            # from both now-free pools, stores spread over two queues
            tail = [(tq, oc2) for tq in range(12, 16) for oc2 in range(2)]
            k = 0
            tailq = [nc.sync, nc.gpsimd, nc.scalar]
            for tq, oc2 in tail:
                for half in range(2):
                    pool = psS if k % 2 == 0 else psF
                    emit_oproj(tq, 2 * oc2 + half, pool=pool, width=256,
                               dq=tailq[k % 3])
                    k += 1

    nc.compile()
    return nc


def _in_maps(x, W_qkv, b_qkv, W_o, b_o):
    x = np.asarray(x, np.float32)
    W_qkv = np.asarray(W_qkv, np.float32)
    b_qkv = np.asarray(b_qkv, np.float32)
    W_o = np.asarray(W_o, np.float32)
    b_o = np.asarray(b_o, np.float32)

    maps = []
    for c in range(N_CORES):
        b, hh = c // 2, c % 2
        rs = slice(hh * OC, (hh + 1) * OC)
        wq = W_qkv[0 * D:1 * D][rs]            # [512, 1024]
        wk = W_qkv[1 * D:2 * D][rs]
        wvv = W_qkv[2 * D:3 * D][rs]
        wqkT = np.concatenate([wq, wk], 0).T   # [1024, 1024]
        bq = b_qkv[0 * D:1 * D][rs]
        bk = b_qkv[1 * D:2 * D][rs]
        bvv = b_qkv[2 * D:3 * D][rs]
        tri = np.triu(np.ones((128, 128), np.float32))
        xTc = np.ascontiguousarray(x[b].T)
        def pack_pn(a):
            # dram row (p*8 + n) holds logical row (n*128 + p): with the
            # "(p n) o" rearrange each partition reads contiguous bytes
            return np.ascontiguousarray(
                a.reshape(8, 128, -1).transpose(1, 0, 2).reshape(a.shape))

        m = {
            "w8": pack_pn(WSCALE * wqkT).astype(F8E4),
            "wvT": pack_pn(np.ascontiguousarray(wvv.T)).astype(BF16),
            "woT": np.ascontiguousarray(W_o[:, rs].T).astype(BF16),
            "bqk": np.ascontiguousarray(
                (WSCALE * np.concatenate([bq, bk]))
                .reshape(2 * OC // 128, 128).T),
            "bv": np.ascontiguousarray(np.tile(bvv.reshape(1, OC), (128, 1))),
            "bo": np.ascontiguousarray(np.tile((0.5 * b_o).reshape(1, D), (128, 1))),
            "tri": tri.astype(BF16),
        }
        for tch in range(2):
            m[f"xt{tch}"] = np.ascontiguousarray(
                xTc[:, tch * 1024:(tch + 1) * 1024]).astype(BF16)
        maps.append(m)
    return maps


def _run(x, W_qkv, b_qkv, W_o, b_o, trace=False, tmpdir=None):
    from concourse.bass_utils import run_bass_kernel_spmd

    if "nc" not in _cache:
        _cache["nc"] = _build()
    res = run_bass_kernel_spmd(
        _cache["nc"], _in_maps(x, W_qkv, b_qkv, W_o, b_o),
        core_ids=list(range(N_CORES)), trace=trace, tmpdir=tmpdir,
    )
    out = np.empty((B, T, D), np.float32)
    for b in range(B):
        out[b] = (res.results[2 * b]["out"].astype(np.float32)
                  + res.results[2 * b + 1]["out"].astype(np.float32))
    return out, res


def kernel(x, W_qkv, b_qkv, W_o, b_o):
    out, _ = _run(x, W_qkv, b_qkv, W_o, b_o, trace=False)
    return out


# revision 46
# speedup vs baseline: 1.1968x; 1.1968x over previous
"""Causal multi-head attention block on 8 Trainium2 NeuronCores.

Problem: x[4,2048,1024] -> QKV proj (16 heads, dh=64) -> causal softmax
attention -> out proj. Sharding: core = (batch, head-half): each core
computes QKV for 8 heads of one batch, flash-style attention for those
heads, and a partial O-projection over its 512 W_o input columns; the
host sums the two partials per batch (tensor-parallel unshard).

Device kernel (identical SPMD program, per-core data). Structure:
  - Q/K projection runs as fp8(e4m3) DoubleRow matmuls: weights are
    host-prescaled by 64 (to clear the e4m3 denormal range) and the
    1/64^2 is folded into the softmax exp scale. Contraction 1024 is
    4 DR matmuls of virtual-K 256 ([128 parts, 2 kd-chunks, .] APs on
    the natural [p, kd, t] tile layout -- no data shuffles). The fp8
    x.T copy is converted on-device by the (early-idle) vector engine.
    V and the O-projection stay bf16 (their error lands on the output).
  - x.T ships as four token-chunk tensors so chunk 0 (all of what the
    first q-chunk needs) streams first; HBM-in is the startup critical
    path (~358 GB/s shared across queues), so the first-needed 3MB
    (W_qk8 | x.T chunk 0 | W_v) leads on three separate queues.
  - scores are computed transposed, S.T[k_tile, q_span] = K.T_blk^T@Q.T.
    Each score call covers ONE k-tile for BOTH heads of a pair (even
    parity head on partitions 0:64, odd on 64:128 -> the two matmuls
    run concurrently as row-tiled PE ops) packed at column 0/512 of one
    2-bank PSUM tile; ScalarE exps both with a single ACTIVATE
    (contiguous for full tiles, 2-segment strided for diagonal ones).
    Diagonal blocks are masked after exp with a 0/1 triangle multiply.
  - O.T[c, q] accumulates with V' stationary: V' has 64 V columns and
    64 ones-columns (parity-dependent order) so the matmul broadcasts
    the softmax denominator for free; normalization is one
    reciprocal_approx_fast + one cross-partition-base multiply.
  - Loop order is J-outer (q-chunk), head-pair-inner. QKV / O-proj
    units pace into the attention phase as PE filler, with the
    deferrable O-projection pushed into the late, ScalarE-bound chunks
    (J=3 attends 4x the keys of J=0). Fill units draw from their own
    2-buf PSUM pool so they never steal a score-pipeline buffer. The
    post-loop O-proj tail runs at 256-column granularity across both
    free PSUM pools with stores spread over two DMA queues.
"""

import numpy as np
import ml_dtypes

BF16 = ml_dtypes.bfloat16
F8E4 = ml_dtypes.float8_e4m3

B, T, D = 4, 2048, 1024
NH, DH = 16, 64
HPC = 8            # heads per core
OC = HPC * DH      # 512: per-core head columns
NT = T // 128      # 16 q/k tiles of 128
ND = D // 128      # 8 d-tiles
N_CORES = 8
WSCALE = 64.0      # host pre-scale on W_qk/b_qk (e4m3 denormal dodge)

_cache = {}


def _build():
    import concourse.mybir as mybir
    import concourse.tile as tile
    from concourse import bacc

    f32 = mybir.dt.float32
    bf16 = mybir.dt.bfloat16
    fp8 = mybir.dt.float8e4
    Exp = mybir.ActivationFunctionType.Exp
    DR = mybir.MatmulPerfMode.DoubleRow

    nc = bacc.Bacc("TRN2", target_bir_lowering=False, debug=False,
                   num_devices=N_CORES)

    # xt halves: 1024-token chunks keep 2KB/partition DMA segments;
    # w8/wv ship (p n)-packed so each partition reads one contiguous run
    xt = [nc.declare_dram_parameter(f"xt{c}", [D, 1024], bf16, isOutput=False)
          for c in range(2)]
    w8 = nc.declare_dram_parameter("w8", [D, 2 * OC], fp8, isOutput=False)
    wv = nc.declare_dram_parameter("wvT", [D, OC], bf16, isOutput=False)
    wo = nc.declare_dram_parameter("woT", [OC, D], bf16, isOutput=False)
    bqk = nc.declare_dram_parameter("bqk", [128, 2 * OC // 128], f32, isOutput=False)
    bv = nc.declare_dram_parameter("bv", [128, OC], f32, isOutput=False)
    bo = nc.declare_dram_parameter("bo", [128, D], f32, isOutput=False)
    tri = nc.declare_dram_parameter("tri", [128, 128], bf16, isOutput=False)
    out = nc.declare_dram_parameter("out", [T, D], bf16, isOutput=True)

    with tile.TileContext(nc) as tc:
        with (
            tc.tile_pool(name="persist", bufs=1) as persist,
            tc.tile_pool(name="pt", bufs=8) as ptp,
            tc.tile_pool(name="dn", bufs=6) as dnp,
            tc.tile_pool(name="ostage", bufs=4) as ostage,
            tc.tile_pool(name="psS", bufs=2, space="PSUM") as psS,
            tc.tile_pool(name="psF", bufs=2, space="PSUM") as psF,
            tc.tile_pool(name="psO", bufs=2, space="PSUM") as psO,
        ):
            # ---- persistent SBUF tensors ----
            XT = persist.tile([128, ND, T], bf16)          # x.T d-tiles (V path)
            X8 = persist.tile([128, ND, T], fp8)           # x.T e4m3 (QK path)
            W8 = persist.tile([128, ND, 2 * OC], fp8)      # 64*W_qk.T e4m3
            WV = persist.tile([128, ND, OC], bf16)
            WO = persist.tile([128, OC // 128, D], bf16)
            BQK = persist.tile([128, 2 * OC // 128], f32)
            BV = persist.tile([128, OC], f32)
            BO = persist.tile([128, D], f32)
            TRI = persist.tile([128, 128], bf16)
            QKT = persist.tile([128, ND, T], bf16)         # [o, t] 64*(Q.T|K.T)
            # V' per head, 128 cols: even h: [V(64) | 1*64]; odd h:
            # [1*64 | V(64)]. O.T rows land on partitions (h%2)*64..+64 and
            # the other 64 rows all become the softmax denominator.
            VP = persist.tile([128, NT, HPC, 128], bf16)
            # attn out.T [c, q] -- one tile per q-chunk J so the deferred
            # O-projection of chunk J never waits on later chunks' writes
            OTJ = [persist.tile([128, OC // 128, 512], bf16, name=f"OTJ{j}")
                   for j in range(4)]

            # warm-up junk matmuls: keep the PE HAM clock-gate warm while
            # the first input DMA chunks stream in; results never read.
            JNK = persist.tile([128, 512], bf16)
            nc.vector.memset(JNK[:], 0.5)
            for g in range(5):
                jps = psF.tile([128, 512], f32, tag="f", name=f"jnk{g}")
                for m in range(8):
                    nc.tensor.matmul(
                        jps[:], lhsT=JNK[:, 0:128], rhs=JNK[:],
                        start=(m == 0), stop=(m == 7),
                    )

            # ---- input DMA: first-needed first, balanced queues ----
            # J=0's critical 4MB (x.T lo, W8, W_v) splits evenly over the
            # sync/gpsimd queues by kd- or partition-halves (both keep the
            # per-partition segments contiguous), so it all lands together
            # ~8us after the DMA queues open; x.T hi and W_o follow.
            # two-queue cascade ordered strictly by first use; the scalar
            # queue carries no input so the exp stream never competes.
            # bv/bo ship pre-broadcast -- a [1,.]->[128,.] broadcast DMA
            # shreds into 128 tiny packets and stalls the shared engines.
            # three queues (per-queue bandwidth caps ~140GB/s; the gpsimd
            # queue consistently opens ~3.5us late, so the J=0-critical
            # x.T-lo halves ride sync+scalar)
            xtr = [t.rearrange("(n p) t -> p n t", p=128) for t in xt]
            w8r = w8.rearrange("(p n) o -> p n o", p=128)
            wvr = wv.rearrange("(p n) o -> p n o", p=128)
            wor = wo.rearrange("(n p) o -> p n o", p=128)
            nc.sync.dma_start(out=XT[:, 0:4, 0:1024], in_=xtr[0][:, 0:4, :])
            nc.sync.dma_start(out=W8[0:64], in_=w8r[0:64])
            nc.sync.dma_start(out=WV[0:64], in_=wvr[0:64])
            nc.sync.dma_start(out=XT[:, :, 1024:2048], in_=xtr[1])
            nc.gpsimd.dma_start(out=XT[:, 4:8, 0:1024], in_=xtr[0][:, 4:8, :])
            nc.gpsimd.dma_start(out=W8[64:128], in_=w8r[64:128])
            nc.gpsimd.dma_start(out=WV[64:128], in_=wvr[64:128])
            nc.gpsimd.dma_start(out=WO[:], in_=wor)
            nc.scalar.dma_start(out=BQK[:], in_=bqk[:, :])
            nc.scalar.dma_start(out=TRI[:], in_=tri[:, :])
            nc.scalar.dma_start(out=BV[:], in_=bv[:, :])
            nc.scalar.dma_start(out=BO[:], in_=bo[:, :])
            # ones-columns of V' via the (otherwise idle) gpsimd engine --
            # these strided memsets cost 3.5us each and would delay the
            # fp8 cast on the vector queue
            nc.gpsimd.memset(VP[:, :, 0:HPC:2, DH:128], 1.0)
            nc.gpsimd.memset(VP[:, :, 1:HPC:2, 0:DH], 1.0)

            def emit_cv(tch):
                # vector-engine bf16 -> e4m3 convert of one x.T t-chunk
                nc.vector.tensor_copy(
                    X8[:, :, tch * 512:(tch + 1) * 512],
                    XT[:, :, tch * 512:(tch + 1) * 512],
                )

            emit_cv(0)

            # ---- QKV / O-proj units (PE filler) ----
            def emit_qk(ot, tch):
                # one [o, t] chunk: [128 o, 512 t] = 64*W_qk @ x.T + 64*b,
                # fp8 DoubleRow: 4 matmuls of virtual-K 256
                ps = psF.tile([128, 512], f32, tag="f",
                              name=f"qk{ot}_{tch}")
                for k in range(4):
                    nc.tensor.matmul(
                        ps[:],
                        lhsT=W8[:, 2 * k:2 * k + 2, ot * 128:(ot + 1) * 128],
                        rhs=X8[:, 2 * k:2 * k + 2, tch * 512:(tch + 1) * 512],
                        start=(k == 0), stop=(k == 3),
                        perf_mode=DR,
                    )
                nc.vector.tensor_scalar_add(
                    QKT[:, ot, tch * 512:(tch + 1) * 512], ps[:],
                    BQK[:, ot:ot + 1],
                )

            def emit_v(tt):
                # one [t, o] tile of V = x @ W_v.T + b, into parity layout
                ps = psF.tile([128, 512], f32, tag="f", name=f"v{tt}")
                for kd in range(ND):
                    nc.tensor.matmul(
                        ps[:],
                        lhsT=XT[:, kd, tt * 128:(tt + 1) * 128],
                        rhs=WV[:, kd, :],
                        start=(kd == 0), stop=(kd == ND - 1),
                    )
                nc.vector.tensor_tensor(
                    out=VP[:, tt, 0:HPC:2, 0:DH],
                    in0=ps[:].rearrange("p (a b) -> p a b", b=DH)[:, 0:HPC:2, :],
                    in1=BV[:].rearrange("p (a b) -> p a b", b=DH)[:, 0:HPC:2, :],
                    op=mybir.AluOpType.add,
                )
                nc.vector.tensor_tensor(
                    out=VP[:, tt, 1:HPC:2, DH:2 * DH],
                    in0=ps[:].rearrange("p (a b) -> p a b", b=DH)[:, 1:HPC:2, :],
                    in1=BV[:].rearrange("p (a b) -> p a b", b=DH)[:, 1:HPC:2, :],
                    op=mybir.AluOpType.add,
                )

            odma = [0]

            def emit_oproj(tq, oc2, pool=None, width=512, beng=None, dq=None):
                # out[tq, oc2-chunk] = O @ WoT + 0.5 b_o (partial over this
                # core's 512 W_o input columns)
                pool = pool or psF
                ps = pool.tile([128, width], f32,
                               tag="s" if pool is psS else "f",
                               name=f"op{tq}_{oc2}_{width}")
                lo = oc2 * width
                OT = OTJ[tq // 4]
                tql = tq % 4
                for ct in range(OC // 128):
                    nc.tensor.matmul(
                        ps[:],
                        lhsT=OT[:, ct, tql * 128:(tql + 1) * 128],
                        rhs=WO[:, ct, lo:lo + width],
                        start=(ct == 0), stop=(ct == OC // 128 - 1),
                    )
                ob = ostage.tile([128, width], bf16, tag="ob")
                (beng or nc.vector).tensor_tensor(
                    out=ob[:], in0=ps[:], in1=BO[:, lo:lo + width],
                    op=mybir.AluOpType.add,
                )
                q = dq or (nc.sync if odma[0] % 2 == 0 else nc.gpsimd)
                odma[0] += 1
                q.dma_start(
                    out=out[tq * 128:(tq + 1) * 128, lo:lo + width],
                    in_=ob[:],
                )

            # prologue (overlaps the input DMA): everything chunk J=0 needs
            for u in [("qk", 0, 0), ("qk", 4, 0), ("v", 0), ("qk", 1, 0),
                      ("qk", 5, 0), ("v", 1), ("qk", 2, 0), ("qk", 6, 0),
                      ("v", 2), ("qk", 3, 0), ("qk", 7, 0), ("v", 3)]:
                if u[0] == "v":
                    emit_v(u[1])
                else:
                    emit_qk(u[1], u[2])

            # fill schedule keyed by global pop index (one pop per score
            # call == one k-tile). (J,hp) starts at pop 16*J*(J+1)/2...
            # J bases 0/16/48/96; (J,hp) at base + hp*(J+1)*4. Deadlines:
            # qk(.,tc) before (tc,hp) scores; v(4J..4J+3) before (J,hp0)
            # PV; oproj(tq) after chunk tq//4 completes -- deferred into
            # the ScalarE-bound J=2/J=3 stretches.
            sched = {
                # J=0 (pops 0..15)
                2: [("cv", 1)],
                4: [("qk", 0, 1)], 6: [("qk", 4, 1)],
                8: [("v", 4)], 10: [("v", 5)], 12: [("v", 6)], 14: [("v", 7)],
                # J=1 (pops 16..47)
                16: [("cv", 2)],
                17: [("qk", 1, 1)], 19: [("qk", 5, 1)],
                20: [("cv", 3)],
                22: [("qk", 2, 1)], 24: [("qk", 6, 1)],
                27: [("qk", 3, 1)], 29: [("qk", 7, 1)],
                32: [("qk", 0, 2)], 34: [("qk", 4, 2)],
                36: [("v", 8)], 38: [("v", 9)], 40: [("v", 10)], 42: [("v", 11)],
                44: [("op", 0, 0)], 46: [("op", 0, 1)],
                # J=2 (pops 48..95): QK/V prefetch only (oproj deferred)
                50: [("qk", 1, 2)], 53: [("qk", 5, 2)],
                62: [("qk", 2, 2)], 65: [("qk", 6, 2)],
                74: [("qk", 3, 2)], 77: [("qk", 7, 2)],
                80: [("v", 12)], 83: [("v", 13)],
                85: [("qk", 0, 3)], 88: [("qk", 4, 3)],
                90: [("v", 14)], 93: [("v", 15)],
                # J=3 (pops 96..159): remaining qk + all deferred oproj
                96: [("op", 1, 0)], 98: [("qk", 1, 3)],
                100: [("op", 1, 1)], 102: [("qk", 5, 3)],
                104: [("op", 2, 0)], 106: [("op", 2, 1)],
                108: [("op", 3, 0)], 110: [("qk", 2, 3)],
                112: [("op", 3, 1)], 114: [("qk", 6, 3)],
                116: [("op", 4, 0)], 118: [("op", 4, 1)],
                120: [("op", 5, 0)], 122: [("qk", 3, 3)],
                124: [("op", 5, 1)], 126: [("qk", 7, 3)],
                128: [("op", 6, 0)], 130: [("op", 6, 1)],
                132: [("op", 7, 0)], 134: [("op", 7, 1)],
                138: [("op", 8, 0)], 141: [("op", 8, 1)],
                144: [("op", 9, 0)], 147: [("op", 9, 1)],
                150: [("op", 10, 0)], 153: [("op", 10, 1)],
                156: [("op", 11, 0)], 158: [("op", 11, 1)],
            }
            giter = [0]

            def pop_fill():
                g = giter[0]
                giter[0] += 1
                for u in sched.get(g, []):
                    if u[0] == "v":
                        emit_v(u[1])
                    elif u[0] == "qk":
                        emit_qk(u[1], u[2])
                    elif u[0] == "cv":
                        emit_cv(u[1])
                    else:
                        emit_oproj(u[1], u[2])

            # ---- attention; O.T accumulated with V' stationary ----
            # per call: one k-tile, both heads (h0 on partitions 0:64, h1
            # on 64:128 -> concurrent row-tiled matmuls), exp'd in one
            # ACTIVATE. av() lags one call behind the exp.
            def st_exp(h0, h1, J, i):
                ps = psS.tile([128, 1024], f32, tag="s",
                              name=f"ps{h0}_{J}_{i}")
                pt = ptp.tile([128, 1024], bf16, tag="p",
                              name=f"pt{h0}_{J}_{i}")
                QT0 = QKT[0:64, h0 // 2, :]
                KT0 = QKT[0:64, 4 + h0 // 2, :]
                QT1 = QKT[64:128, h1 // 2, :]
                KT1 = QKT[64:128, 4 + h1 // 2, :]
                qlo = max(J * 512, i * 128)
                span = (J + 1) * 512 - qlo
                nc.tensor.matmul(
                    ps[:, 0:span],
                    lhsT=KT0[:, i * 128:(i + 1) * 128],
                    rhs=QT0[:, qlo:qlo + span],
                    start=True, stop=True,
                )
                nc.tensor.matmul(
                    ps[:, 512:512 + span],
                    lhsT=KT1[:, i * 128:(i + 1) * 128],
                    rhs=QT1[:, qlo:qlo + span],
                    start=True, stop=True,
                )
                # QKT holds 64*(Q|K): scores are 4096x -> fold into scale
                sc = 0.125 / (WSCALE * WSCALE)
                if span == 512:
                    nc.scalar.activation(
                        out=pt[:], in_=ps[:], func=Exp, scale=sc)
                else:
                    psv = ps[:].rearrange("p (a b) -> p a b", b=512)[:, :, 0:span]
                    ptv = pt[:].rearrange("p (a b) -> p a b", b=512)[:, :, 0:span]
                    nc.scalar.activation(
                        out=ptv, in_=psv, func=Exp, scale=sc)
                return (pt, i, qlo, span)

            def av2(h0, h1, J, unit, otr0, otr1):
                pt, i, qlo, span = unit
                if i >= 4 * J:  # diagonal: zero upper triangle
                    nc.vector.tensor_tensor(
                        out=pt[:, 0:128], in0=pt[:, 0:128], in1=TRI[:],
                        op=mybir.AluOpType.mult,
                    )
                    nc.vector.tensor_tensor(
                        out=pt[:, 512:640], in0=pt[:, 512:640], in1=TRI[:],
                        op=mybir.AluOpType.mult,
                    )
                qloc = qlo - J * 512
                nc.tensor.matmul(
                    otr0[:, qloc:512],
                    lhsT=VP[:, i, h0, :],
                    rhs=pt[:, 0:span],
                    start=(i == 0), stop=(i == 4 * J + 3),
                )
                nc.tensor.matmul(
                    otr1[:, qloc:512],
                    lhsT=VP[:, i, h1, :],
                    rhs=pt[:, 512:512 + span],
                    start=(i == 0), stop=(i == 4 * J + 3),
                )

            def normalize(h, J, otr, last=False):
                # O.T rows (base prow) times 1/den rows (base drow; all 64
                # denominator rows are identical by construction)
                prow = (h % 2) * 64
                drow = 64 - prow
                rd = dnp.tile([128, 512], f32, tag="d", name=f"rd{h}_{J}")
                if drow == 0:
                    nc.vector.reciprocal_approx_fast(
                        rd[0:64, :], otr[0:64, :])
                else:
                    # reciprocal_approx_fast needs its source at base 0;
                    # the final chunk's copy runs on the (then-idle)
                    # scalar engine so the tail isn't DVE-serialized
                    rdc = dnp.tile([128, 512], f32, tag="dc",
                                   name=f"rdc{h}_{J}")
                    nc.vector.tensor_copy(
                        rdc[0:64, :], otr[drow:drow + 64, :])
                    nc.vector.reciprocal_approx_fast(
                        rd[0:64, :], rdc[0:64, :])
                nc.vector.tensor_tensor(
                    out=OTJ[J][prow:prow + 64, h // 2, :],
                    in0=otr[prow:prow + 64, :],
                    in1=rd[0:64, :],
                    op=mybir.AluOpType.mult,
                )

            for J in range(T // 512):
                for hp in range(HPC // 2):
                    h0, h1 = 2 * hp, 2 * hp + 1
                    otr0 = psO.tile([128, 512], f32, tag="o",
                                    name=f"otr{h0}_{J}")
                    otr1 = psO.tile([128, 512], f32, tag="o",
                                    name=f"otr{h1}_{J}")
                    prev = None
                    for i in range(4 * J + 4):
                        unit = st_exp(h0, h1, J, i)
                        pop_fill()
                        if prev is not None:
                            av2(h0, h1, J, prev, otr0, otr1)
                        prev = unit
                    av2(h0, h1, J, prev, otr0, otr1)
                    # h0's normalize (DVE chain) overlaps h1's PV matmuls
                    normalize(h0, J, otr0)
                    normalize(h1, J, otr1, last=(J == 3 and hp == 3))

            # tail: remaining O-projection at 256-col granularity, psum
            # from both now-free pools, stores spread over two queues
            tail = [(tq, oc2) for tq in range(12, 16) for oc2 in range(2)]
            k = 0
            for tq, oc2 in tail:
                for half in range(2):
                    pool = psS if k % 2 == 0 else psF
                    emit_oproj(tq, 2 * oc2 + half, pool=pool, width=256)
                    k += 1

    nc.compile()
    return nc


def _in_maps(x, W_qkv, b_qkv, W_o, b_o):
    x = np.asarray(x, np.float32)
    W_qkv = np.asarray(W_qkv, np.float32)
    b_qkv = np.asarray(b_qkv, np.float32)
    W_o = np.asarray(W_o, np.float32)
    b_o = np.asarray(b_o, np.float32)

    maps = []
    for c in range(N_CORES):
        b, hh = c // 2, c % 2
        rs = slice(hh * OC, (hh + 1) * OC)
        wq = W_qkv[0 * D:1 * D][rs]            # [512, 1024]
        wk = W_qkv[1 * D:2 * D][rs]
        wvv = W_qkv[2 * D:3 * D][rs]
        wqkT = np.concatenate([wq, wk], 0).T   # [1024, 1024]
        bq = b_qkv[0 * D:1 * D][rs]
        bk = b_qkv[1 * D:2 * D][rs]
        bvv = b_qkv[2 * D:3 * D][rs]
        tri = np.triu(np.ones((128, 128), np.float32))
        xTc = np.ascontiguousarray(x[b].T)
        def pack_pn(a):
            # dram row (p*8 + n) holds logical row (n*128 + p): with the
            # "(p n) o" rearrange each partition reads contiguous bytes
            return np.ascontiguousarray(
                a.reshape(8, 128, -1).transpose(1, 0, 2).reshape(a.shape))

        m = {
            "w8": pack_pn(WSCALE * wqkT).astype(F8E4),
            "wvT": pack_pn(np.ascontiguousarray(wvv.T)).astype(BF16),
            "woT": np.ascontiguousarray(W_o[:, rs].T).astype(BF16),
            "bqk": np.ascontiguousarray(
                (WSCALE * np.concatenate([bq, bk]))
                .reshape(2 * OC // 128, 128).T),
            "bv": np.ascontiguousarray(np.tile(bvv.reshape(1, OC), (128, 1))),
            "bo": np.ascontiguousarray(np.tile((0.5 * b_o).reshape(1, D), (128, 1))),
            "tri": tri.astype(BF16),
        }
        for tch in range(2):
            m[f"xt{tch}"] = np.ascontiguousarray(
                xTc[:, tch * 1024:(tch + 1) * 1024]).astype(BF16)
        maps.append(m)
    return maps


def _run(x, W_qkv, b_qkv, W_o, b_o, trace=False, tmpdir=None):
    from concourse.bass_utils import run_bass_kernel_spmd

    if "nc" not in _cache:
        _cache["nc"] = _build()
    res = run_bass_kernel_spmd(
        _cache["nc"], _in_maps(x, W_qkv, b_qkv, W_o, b_o),
        core_ids=list(range(N_CORES)), trace=trace, tmpdir=tmpdir,
    )
    out = np.empty((B, T, D), np.float32)
    for b in range(B):
        out[b] = (res.results[2 * b]["out"].astype(np.float32)
                  + res.results[2 * b + 1]["out"].astype(np.float32))
    return out, res


def kernel(x, W_qkv, b_qkv, W_o, b_o):
    out, _ = _run(x, W_qkv, b_qkv, W_o, b_o, trace=False)
    return out
